# revision 8
# baseline (speedup 1.0000x reference)
"""Causal GQA self-attention (RoPE + QK-RMSNorm) Trainium2 kernel.

Sharding: 8 cores = batch (2) x kv-head-group (4). Each core computes, for
its (batch b, kv-group g): the 4 query heads + 1 kv head of that group,
causal attention over the full sequence, and a partial output projection
y_bg = O_g @ W_O[rows of group g]. Host sums the 4 partials per batch.

Device layout is "transposed" throughout: activations live as [feature,
token] so every matmul contracts over the partition axis with 512-wide
moving operands. Matmuls run in bf16 (f32 PSUM accumulation); softmax,
RoPE and RMS statistics stay f32.
"""

import sys
import types

import numpy as np
import ml_dtypes

import concourse.bass as bass  # noqa: F401
import concourse.tile as tile
from concourse import bacc, mybir
from concourse import bass_utils

BF16 = mybir.dt.bfloat16
F32 = mybir.dt.float32
NPBF16 = ml_dtypes.bfloat16

P = 128          # partitions == head_dim
HALF = 64        # rope half-dim
TB = 512         # t-block (psum free width)
S = 128          # s-tile (score partition block)
EPS = float(np.finfo(np.float32).eps)
NEG = -1e9


def _build(T, C, G, n_devices=8):
    """Build the single-core SPMD program. T seq len, C model dim, G q-heads."""
    NB = T // TB         # t-blocks
    NC = C // P          # contraction tiles for projections
    SPB = TB // S        # s-tiles per t-block (4)
    NS = T // S          # s-tiles total
    DQ = G * P

    nc = bacc.Bacc("TRN2", target_bir_lowering=False, debug=False,
                   num_devices=n_devices)

    xT = nc.dram_tensor("xT", [C, T], BF16, kind="ExternalInput").ap()
    wq = nc.dram_tensor("wq", [C, DQ], BF16, kind="ExternalInput").ap()
    wk = nc.dram_tensor("wk", [C, P], BF16, kind="ExternalInput").ap()
    wv = nc.dram_tensor("wv", [C, P], BF16, kind="ExternalInput").ap()
    wo = nc.dram_tensor("wo", [DQ, C], BF16, kind="ExternalInput").ap()
    ccd = nc.dram_tensor("cc", [P, T], F32, kind="ExternalInput").ap()
    ssd = nc.dram_tensor("ss", [P, T], F32, kind="ExternalInput").ap()
    y = nc.dram_tensor("y", [T, C], F32, kind="ExternalOutput").ap()

    # additive causal mask for the SPB diagonal offsets: valid iff S*o+p <= f
    pp = np.arange(P)[:, None]
    ff = np.arange(TB)[None, :]
    mb_np = np.stack([np.where(S * o + pp <= ff, 0.0, NEG) for o in range(SPB)])
    mb_d = nc.inline_tensor(mb_np.astype(NPBF16), "mbias").ap()
    idn_d = nc.inline_tensor(np.eye(P, dtype=NPBF16), "idn").ap()
    onesf_d = nc.inline_tensor(np.ones((P, 1), np.float32), "onesf").ap()
    onesb_d = nc.inline_tensor(np.ones((P, 1), NPBF16), "onesb").ap()

    with tile.TileContext(nc) as tc:
        with (
            tc.tile_pool(name="const", bufs=1) as const,
            tc.tile_pool(name="resid", bufs=1) as resid,
            tc.tile_pool(name="xp", bufs=2) as xp,
            tc.tile_pool(name="work", bufs=3) as work,
            tc.tile_pool(name="rows", bufs=3) as rows,
            tc.tile_pool(name="pp", bufs=3) as ppool,
            tc.tile_pool(name="yp", bufs=3) as yp,
            tc.tile_pool(name="ps_mm", bufs=4, space="PSUM") as ps_mm,
            tc.tile_pool(name="ps_o", bufs=2, space="PSUM") as ps_o,
            tc.tile_pool(name="ps_r", bufs=2, space="PSUM") as ps_r,
        ):
            # ---- constants into SBUF ----
            mb_sb = const.tile([P, SPB, TB], BF16, tag="mb")
            nc.sync.dma_start(mb_sb, mb_d.rearrange("o p f -> p o f"))
            idn = const.tile([P, P], BF16, tag="idn")
            nc.sync.dma_start(idn, idn_d)
            ones_f = const.tile([P, 1], F32, tag="onesf")
            nc.sync.dma_start(ones_f, onesf_d)
            ones_b = const.tile([P, 1], BF16, tag="onesb")
            nc.sync.dma_start(ones_b, onesb_d)
            cc_sb = const.tile([P, T], F32, tag="cc")
            nc.sync.dma_start(cc_sb, ccd)
            ss_sb = const.tile([P, T], F32, tag="ss")
            nc.sync.dma_start(ss_sb, ssd)
            wq_sb = const.tile([P, NC, DQ], BF16, tag="wq")
            nc.sync.dma_start(wq_sb, wq.rearrange("(ci p) j -> p ci j", p=P))
            wk_sb = const.tile([P, NC, P], BF16, tag="wk")
            nc.sync.dma_start(wk_sb, wk.rearrange("(ci p) j -> p ci j", p=P))
            wv_sb = const.tile([P, NC, P], BF16, tag="wv")
            nc.sync.dma_start(wv_sb, wv.rearrange("(ci p) j -> p ci j", p=P))
            wo_sb = const.tile([P, G, C], BF16, tag="wo")
            nc.sync.dma_start(wo_sb, wo.rearrange("(g p) c -> p g c", p=P))
            eps_q = const.tile([P, 1], F32, tag="epsq")
            nc.vector.memset(eps_q, P * EPS)
            eps_k = const.tile([P, 1], F32, tag="epsk")
            nc.vector.memset(eps_k, EPS)

            # ---- resident per-block activations (fine-grained for deps) ----
            qT = [[resid.tile([P, TB], BF16, tag=f"qT{h}_{j}", name=f"qT{h}_{j}") for j in range(NB)]
                  for h in range(G)]
            kT = [resid.tile([P, TB], BF16, tag=f"kT{j}", name=f"kT{j}") for j in range(NB)]
            vN = [resid.tile([P, P], BF16, tag=f"v{si}", name=f"v{si}") for si in range(NS)]
            oT = [[resid.tile([P, TB], BF16, tag=f"oT{h}_{j}", name=f"oT{h}_{j}") for j in range(NB)]
                  for h in range(G)]

            def rope_rms(ps, dest, j, is_q):
                """psum [d=128, t=TB] f32 -> dest sbuf bf16 (roped + rms-normed;
                q additionally folded with the 1/sqrt(d) score scale)."""
                blk = slice(j * TB, (j + 1) * TB)
                qp = work.tile([P, TB], F32, tag="qpre", bufs=2)
                nc.any.tensor_copy(qp, ps)
                rot = work.tile([P, TB], F32, tag="rot", bufs=2)
                nc.sync.dma_start(rot[0:HALF, :], qp[HALF:P, :])
                nc.sync.dma_start(rot[HALF:P, :], qp[0:HALF, :])
                a = work.tile([P, TB], F32, tag="ra", bufs=2)
                nc.vector.tensor_mul(a, qp, cc_sb[:, blk])
                m = work.tile([P, TB], F32, tag="rm", bufs=2)
                nc.gpsimd.tensor_mul(m, rot, ss_sb[:, blk])
                qr = work.tile([P, TB], F32, tag="qr")
                nc.vector.tensor_add(qr, a, m)
                q2 = work.tile([P, TB], F32, tag="ra", bufs=2, name="q2")
                nc.scalar.activation(q2, qr, mybir.ActivationFunctionType.Square)
                srow = ps_r.tile([1, TB], F32, tag="row")
                nc.tensor.matmul(srow, ones_f, q2, start=True, stop=True)
                sq = rows.tile([1, TB], F32, tag="sq")
                if is_q:   # 1/sqrt(sum+d*eps) == rsqrt(mean+eps)/sqrt(d)
                    nc.scalar.activation(sq, srow,
                                         mybir.ActivationFunctionType.Sqrt,
                                         bias=eps_q[:1, :], scale=1.0)
                else:
                    nc.scalar.activation(sq, srow,
                                         mybir.ActivationFunctionType.Sqrt,
                                         bias=eps_k[:1, :], scale=1.0 / P)
                nc.vector.reciprocal(sq, sq)
                inv = sq
                invb = work.tile([P, TB], F32, tag="invb", bufs=2)
                nc.gpsimd.partition_broadcast(invb, inv)
                nc.vector.tensor_mul(dest, qr, invb)

            # ---- phase 1: QKV projections + rope + rms ----
            for j in range(NB):
                xs = xp.tile([P, NC, TB], BF16, tag="xs")
                nc.sync.dma_start(
                    xs, xT[:, j * TB:(j + 1) * TB].rearrange(
                        "(ci p) t -> p ci t", p=P))
                for h in range(G):
                    ps = ps_mm.tile([P, TB], F32, tag="mm")
                    for ci in range(NC):
                        nc.tensor.matmul(ps, wq_sb[:, ci, h * P:(h + 1) * P],
                                         xs[:, ci, :],
                                         start=(ci == 0), stop=(ci == NC - 1))
                    rope_rms(ps, qT[h][j], j, True)
                ps = ps_mm.tile([P, TB], F32, tag="mm")
                for ci in range(NC):
                    nc.tensor.matmul(ps, wk_sb[:, ci, :], xs[:, ci, :],
                                     start=(ci == 0), stop=(ci == NC - 1))
                rope_rms(ps, kT[j], j, False)
                ps = ps_mm.tile([P, TB], F32, tag="mm")
                for ci in range(NC):
                    nc.tensor.matmul(ps, wv_sb[:, ci, :], xs[:, ci, :],
                                     start=(ci == 0), stop=(ci == NC - 1))
                vp = work.tile([P, TB], BF16, tag="vp", bufs=2)
                nc.any.tensor_copy(vp, ps)
                for k4 in range(SPB):
                    pt = ps_mm.tile([P, P], BF16, tag="mm")
                    nc.tensor.transpose(pt, vp[:, k4 * P:(k4 + 1) * P], idn)
                    nc.any.tensor_copy(vN[j * SPB + k4], pt)

            # ---- phase 2: causal attention (scores^T -> exp -> AV) ----
            for j in range(NB):
                for h in range(G):
                    ns = (j + 1) * SPB
                    oac = ps_o.tile([P, TB], F32, tag="oac")
                    rrow = ps_r.tile([1, TB], F32, tag="row")
                    for si in range(ns):
                        diag_o = si - j * SPB
                        sps = ps_mm.tile([P, TB], F32, tag="mm")
                        nc.tensor.matmul(sps, kT[si // SPB][:, (si % SPB) * S:
                                                            (si % SPB + 1) * S],
                                         qT[h][j],
                                         start=True, stop=(diag_o < 0))
                        if diag_o >= 0:
                            nc.tensor.matmul(sps, idn, mb_sb[:, diag_o, :],
                                             start=False, stop=True)
                        pex = ppool.tile([P, TB], BF16, tag="p")
                        nc.scalar.activation(pex, sps,
                                             mybir.ActivationFunctionType.Exp)
                        nc.tensor.matmul(oac, vN[si], pex,
                                         start=(si == 0), stop=(si == ns - 1))
                        nc.tensor.matmul(rrow, ones_b, pex,
                                         start=(si == 0), stop=(si == ns - 1))
                    rinv = rows.tile([1, TB], F32, tag="rinv")
                    nc.vector.reciprocal(rinv, rrow)
                    rb = work.tile([P, TB], F32, tag="rb", bufs=2)
                    nc.gpsimd.partition_broadcast(rb, rinv)
                    nc.vector.tensor_mul(oT[h][j], oac, rb)

            # ---- phase 3: output projection (partial; host sums groups) ----
            TPB = TB // P  # t-tiles per block
            NYB = C // TB  # y column blocks
            for ti in range(T // P):
                for yb in range(NYB):
                    yps = ps_mm.tile([P, TB], F32, tag="mm")
                    for h in range(G):
                        nc.tensor.matmul(
                            yps,
                            oT[h][ti // TPB][:, (ti % TPB) * P:(ti % TPB + 1) * P],
                            wo_sb[:, h, yb * TB:(yb + 1) * TB],
                            start=(h == 0), stop=(h == G - 1))
                    ys = yp.tile([P, TB], F32, tag="ys")
                    nc.any.tensor_copy(ys, yps)
                    nc.sync.dma_start(
                        y[ti * P:(ti + 1) * P, yb * TB:(yb + 1) * TB], ys)

    nc.compile()
    return nc


_NC_CACHE = {}


def _get_nc(T, C, G):
    key = (T, C, G)
    if key not in _NC_CACHE:
        _NC_CACHE[key] = _build(T, C, G)
    return _NC_CACHE[key]


def _host_prep(x, cos, sin, W_Q, W_K, W_V, W_O, G):
    """Build the 8 per-core input maps (batch-major, then kv-group)."""
    B, T, C = x.shape
    n_kv = W_K.shape[1] // P
    cosT = np.ascontiguousarray(cos.reshape(T, HALF).T.astype(np.float32))
    sinT = np.ascontiguousarray(sin.reshape(T, HALF).T.astype(np.float32))
    cc = np.concatenate([cosT, cosT], axis=0)            # [128, T]
    ss = np.concatenate([sinT, -sinT], axis=0)           # [128, T]
    in_maps = []
    for b in range(B):
        xTb = np.ascontiguousarray(x[b].T).astype(NPBF16)
        for g in range(n_kv):
            in_maps.append({
                "xT": xTb,
                "wq": np.ascontiguousarray(
                    W_Q[:, g * G * P:(g + 1) * G * P]).astype(NPBF16),
                "wk": np.ascontiguousarray(
                    W_K[:, g * P:(g + 1) * P]).astype(NPBF16),
                "wv": np.ascontiguousarray(
                    W_V[:, g * P:(g + 1) * P]).astype(NPBF16),
                "wo": np.ascontiguousarray(
                    W_O[g * G * P:(g + 1) * G * P, :]).astype(NPBF16),
                "cc": cc,
                "ss": ss,
            })
    return in_maps


def kernel(x, cos, sin, W_Q, W_K, W_V, W_O):
    B, T, C = x.shape
    n_kv = W_K.shape[1] // P
    n_head = W_Q.shape[1] // P
    G = n_head // n_kv
    x = np.asarray(x, dtype=np.float32)
    nc = _get_nc(T, C, G)
    in_maps = _host_prep(x, np.asarray(cos), np.asarray(sin),
                         np.asarray(W_Q), np.asarray(W_K), np.asarray(W_V),
                         np.asarray(W_O), G)
    res = bass_utils.run_bass_kernel_spmd(
        nc, in_maps, core_ids=list(range(B * n_kv)))
    out = np.zeros((B, T, C), dtype=np.float32)
    for b in range(B):
        for g in range(n_kv):
            out[b] += res.results[b * n_kv + g]["y"]
    return out
